# revision 1
# baseline (speedup 1.0000x reference)
"""Trainium2 Bass kernel for per-sample segment-mean + tiny GEMM.

Reference computation (per sample b):
    sums[w]  = segment_sum(x[b], word_ids[b])           # [512, 1024]
    cnt[w]   = segment_sum(ones, word_ids[b])           # [512]
    means    = sums / max(cnt, 1)
    out[b]   = means[word_ids[b]] @ W + b               # [2048, 3]

Device strategy (data parallel: 2 samples per core, 8 cores):
  The tensor engine contracts over the partition axis, so everything is
  phrased as token-contraction with tokens kept on partitions (x is never
  transposed; it streams through the PE as the moving operand in its
  natural [token, H] layout):

  A. Per 128-token chunk: ind[s,w] = (iota[w] == word_id[s]) via DVE
     tensor_scalar(is_equal).  sums[wblock] += ind[:,wblock].T @ x_chunk
     accumulated in PSUM over chunks (word blocks of 128; the per-chunk
     touched word blocks are computed on the host from the sorted ids and
     unioned across samples so the program is SPMD-identical).
  B. sums [w,1024] -> PE-transpose (128x128 tiles) -> sumsT [h,w];
     ysumT[c,w] = sum_h W[h,c] * sumsT[h,w] via 8 accumulated matmuls.
  C. Gather back: wib = broadcast(word_ids) via ones-matmul,
     indT[w,s] = is_equal(wib, iota_part) (also yields counts via free-dim
     reduce), ymean[w,c] = ysumT.T[w,c] * (1/max(cnt,1)) per-partition,
     outT[c,s] = sum_w ymean[w,c] * indT[w,s] + bias via accumulated
     matmuls.  Host transposes [3,2048] -> [2048,3] per sample.

  All big matmuls use float32r (full fp32 bits, 1 cycle/row at N>=512).
  HBM traffic = x read once (~16.8 MB/core) => ~47 us/core roofline.
"""

import numpy as np

import concourse.bass as bass
import concourse.bacc as bacc
import concourse.mybir as mybir
import concourse.tile as tile
from concourse.bass_utils import run_bass_kernel_spmd
from concourse.masks import make_identity

B, S, H, C = 16, 2048, 1024, 3
NW = 512
P = 128
N_CORES = 8
SPC = B // N_CORES          # samples per core
NCH = S // P                # 128-token chunks per sample
NST = S // 512              # 512-token strips per sample
NWB = NW // P               # word blocks
NHC = H // P                # h chunks
F32 = mybir.dt.float32
F32R = mybir.dt.float32r

_CACHE = {}
TRACE = False          # set by test harness to capture an NTFF profile
LAST_RESULTS = None    # BassKernelResults of the most recent run


def _build_maps(word_ids):
    """Per-chunk / per-strip touched word-block sets, unioned across all
    samples so the same program is valid on every core (untouched blocks
    just accumulate zeros)."""
    chunk_wbs = [set() for _ in range(NCH)]
    strip_wbs = [set() for _ in range(NST)]
    for bi in range(B):
        for ci in range(NCH):
            seg = word_ids[bi, ci * P:(ci + 1) * P]
            lo, hi = int(seg.min()) // P, int(seg.max()) // P
            chunk_wbs[ci].update(range(lo, hi + 1))
        for si in range(NST):
            seg = word_ids[bi, si * 512:(si + 1) * 512]
            lo, hi = int(seg.min()) // P, int(seg.max()) // P
            strip_wbs[si].update(range(lo, hi + 1))
    # ensure every word block is touched by at least one chunk so its sums
    # region is always initialized (never-gathered garbage would still
    # poison downstream matmuls as NaN otherwise)
    seen = set().union(*chunk_wbs)
    for wb in range(NWB):
        if wb not in seen:
            chunk_wbs[0].add(wb)
    chunk_wbs = [sorted(sset) for sset in chunk_wbs]
    strip_wbs = [sorted(sset) for sset in strip_wbs]
    first_ch = {wb: min(ci for ci in range(NCH) if wb in chunk_wbs[ci])
                for wb in range(NWB)}
    last_ch = {wb: max(ci for ci in range(NCH) if wb in chunk_wbs[ci])
               for wb in range(NWB)}
    return chunk_wbs, strip_wbs, first_ch, last_ch


def _build_program(maps):
    chunk_wbs, strip_wbs, first_ch, last_ch = maps
    nc = bacc.Bacc(
        "TRN2",
        target_bir_lowering=False,
        debug=False,
        enable_asserts=False,
        num_devices=N_CORES,
    )

    # f32r DRAM tensors: fp32r memory bytes are identical to fp32 (verified
    # bit-exact on HW vs the gpsimd cast path), so plain HWDGE loads work.
    xc = nc.dram_tensor("xc", [SPC * S, H], F32R, kind="ExternalInput").ap()
    wic = nc.dram_tensor("wic", [P, SPC * NCH], F32, kind="ExternalInput").ap()
    wir = nc.dram_tensor("wir", [1, SPC * S], F32R, kind="ExternalInput").ap()
    iota = nc.dram_tensor("iota", [P, NW], F32, kind="ExternalInput").ap()
    iotap = nc.dram_tensor("iotap", [P, NWB], F32, kind="ExternalInput").ap()
    wt = nc.dram_tensor("wt", [P, NHC * C], F32R, kind="ExternalInput").ap()
    bb = nc.dram_tensor("bb", [4, 1], F32, kind="ExternalInput").ap()
    onesd = nc.dram_tensor("onesd", [1, P], F32R, kind="ExternalInput").ap()
    yout = nc.dram_tensor("yout", [SPC, C, S], F32, kind="ExternalOutput").ap()

    XCH = 2                     # 128-token chunks per x DMA (1 MB loads)

    with tile.TileContext(nc) as tc:
        with (
            tc.tile_pool(name="pp_sums", bufs=2, space="PSUM") as pp_sums,
            tc.tile_pool(name="pp_aux", bufs=3, space="PSUM") as pp_aux,
            tc.tile_pool(name="pp_warm", bufs=1, space="PSUM") as pp_warm,
            tc.tile_pool(name="pl_x", bufs=4) as pl_x,
            tc.tile_pool(name="pl_ind", bufs=6) as pl_ind,
            tc.tile_pool(name="pl_sums", bufs=3) as pl_sums,
            tc.tile_pool(name="pl_sumsT", bufs=2 * NHC) as pl_sumsT,
            tc.tile_pool(name="pl_indT", bufs=14) as pl_indT,
            tc.tile_pool(name="pl_small", bufs=4) as pl_small,
            tc.tile_pool(name="pl_out", bufs=2) as pl_out,
            tc.tile_pool(name="pl_const", bufs=1) as pl_const,
        ):
            XG = NCH // XCH            # x DMA groups per sample
            x_tiles = {}

            def load_x(s, g):
                t = pl_x.tile([P, XCH * H], F32R, tag="x", name=f"x_{s}_{g}")
                r0 = s * S + g * XCH * P
                nc.gpsimd.dma_start(
                    out=t[:].rearrange("p (n h) -> p n h", n=XCH),
                    in_=xc[r0:r0 + XCH * P, :].rearrange(
                        "(n p) h -> p n h", p=P),
                )
                x_tiles[(s, g)] = t

            # prefetch the first x tiles before anything else so the HBM
            # stream starts at t=0
            load_x(0, 0)
            load_x(0, 1)

            # --- constants, loaded once ---
            wic_sb = pl_const.tile([P, SPC * NCH], F32, tag="wic")
            nc.sync.dma_start(out=wic_sb[:], in_=wic[:])
            wir_sb = pl_const.tile([1, SPC * S], F32R, tag="wir")
            nc.sync.dma_start(out=wir_sb[:], in_=wir[:])
            iota_sb = pl_const.tile([P, NW], F32, tag="iota")
            nc.sync.dma_start(out=iota_sb[:], in_=iota[:])
            iotap_sb = pl_const.tile([P, NWB], F32, tag="iotap")
            nc.sync.dma_start(out=iotap_sb[:], in_=iotap[:])
            wt_sb = pl_const.tile([P, NHC * C], F32R, tag="wt")
            nc.sync.dma_start(out=wt_sb[:], in_=wt[:])
            bb_sb = pl_const.tile([4, 1], F32, tag="bb")
            nc.sync.dma_start(out=bb_sb[:], in_=bb[:])
            ident = pl_const.tile([P, P], F32, tag="ident")
            make_identity(nc, ident[:])
            ones_sb = pl_const.tile([1, P], F32R, tag="ones")
            nc.sync.dma_start(out=ones_sb[:], in_=onesd[:])

            # PE warm-up: ~4us of junk matmuls on constants so the HAM
            # un-throttles the PE clock while the first x tile is in flight.
            warm = pp_warm.tile([P, 512], F32, tag="warm", name="warm")
            for _ in range(20):
                nc.tensor.matmul(out=warm[:], lhsT=ones_sb[:],
                                 rhs=wir_sb[0:1, 0:512], start=True, stop=True)

            for s in range(SPC):
                # ---------------- Phase A: segment sums ----------------
                sums_ps = {}
                sumsT_sb = [pl_sumsT.tile([P, NW], F32R, tag="sumsT",
                                          name=f"sumsT_{s}_{hc}")
                            for hc in range(NHC)]
                for ci in range(NCH):
                    if ci % XCH == 0:
                        g = ci // XCH
                        if (s, g) not in x_tiles:
                            load_x(s, g)
                    x4 = x_tiles[(s, ci // XCH)]
                    xv = x4[:, (ci % XCH) * H:(ci % XCH + 1) * H]
                    lo, hi = chunk_wbs[ci][0], chunk_wbs[ci][-1]
                    nb = hi - lo + 1
                    ind = pl_ind.tile([P, 2 * P], F32R, tag="ind",
                                      name=f"ind_{s}_{ci}")
                    nc.vector.tensor_scalar(
                        out=ind[:, 0:nb * P],
                        in0=iota_sb[:, lo * P:(hi + 1) * P],
                        scalar1=wic_sb[:, s * NCH + ci:s * NCH + ci + 1],
                        scalar2=None,
                        op0=mybir.AluOpType.is_equal,
                    )
                    for wb in chunk_wbs[ci]:
                        if ci == first_ch[wb]:
                            sums_ps[wb] = pp_sums.tile(
                                [P, H], F32, tag="sums",
                                name=f"sums_{s}_{wb}")
                        for hh in range(2):
                            nc.tensor.matmul(
                                out=sums_ps[wb][:, hh * 512:(hh + 1) * 512],
                                lhsT=ind[:, (wb - lo) * P:(wb - lo + 1) * P],
                                rhs=xv[:, hh * 512:(hh + 1) * 512],
                                start=(ci == first_ch[wb]),
                                stop=(ci == last_ch[wb]),
                            )
                    # HAM heartbeat: one junk matmul per chunk keeps the
                    # PE activity monitor from re-throttling the clock
                    nc.tensor.matmul(out=warm[:], lhsT=ones_sb[:],
                                     rhs=wir_sb[0:1, 0:512],
                                     start=True, stop=True)
                    # A.5: retire finished word blocks: evac + transpose
                    for wb in list(sums_ps.keys()):
                        if ci != last_ch[wb]:
                            continue
                        sums_sb = pl_sums.tile([P, H], F32, tag="sums_sb",
                                               name=f"sums_sb_{s}_{wb}")
                        nc.scalar.copy(out=sums_sb[:], in_=sums_ps[wb][:])
                        del sums_ps[wb]
                        for hc in range(NHC):
                            tp = pp_aux.tile([P, P], F32, tag="aux",
                                            name=f"tp_{s}_{wb}_{hc}")
                            nc.tensor.transpose(
                                out=tp[:],
                                in_=sums_sb[:, hc * P:(hc + 1) * P],
                                identity=ident[:],
                            )
                            if hc % 2 == 0:
                                nc.vector.tensor_copy(
                                    out=sumsT_sb[hc][:, wb * P:(wb + 1) * P],
                                    in_=tp[:])
                            else:
                                nc.scalar.copy(
                                    out=sumsT_sb[hc][:, wb * P:(wb + 1) * P],
                                    in_=tp[:])

                # ---------------- Phase B: GEMM over H ----------------
                ysumT_ps = pp_aux.tile([4, NW], F32, tag="aux",
                                         name=f"ysumT_ps_{s}")
                for hc in range(NHC):
                    nc.tensor.matmul(
                        out=ysumT_ps[0:C, :],
                        lhsT=wt_sb[:, hc * C:(hc + 1) * C],
                        rhs=sumsT_sb[hc][:],
                        start=(hc == 0),
                        stop=(hc == NHC - 1),
                    )
                ysumT_sb = pl_small.tile([4, NW], F32, tag="ysumT",
                                         name=f"ysumT_sb_{s}")
                nc.vector.memset(ysumT_sb[:], 0.0)
                nc.scalar.copy(out=ysumT_sb[0:C, :], in_=ysumT_ps[0:C, :])

                # ---------------- Phase C: counts, means, gather ------
                cnt_sb = pl_small.tile([P, NST * NWB], F32, tag="cnt",
                                       name=f"cnt_{s}")
                nc.vector.memset(cnt_sb[:], 0.0)
                indT_sb = {}
                for si in range(NST):
                    wib = pp_aux.tile([P, 512], F32, tag="aux",
                                     name=f"wib_{s}_{si}")
                    nc.tensor.matmul(
                        out=wib[:],
                        lhsT=ones_sb[:],
                        rhs=wir_sb[0:1, s * S + si * 512:s * S + (si + 1) * 512],
                        start=True,
                        stop=True,
                    )
                    for wb in strip_wbs[si]:
                        it = pl_indT.tile([P, 512], F32R, tag="indT",
                                          name=f"indT_{s}_{si}_{wb}")
                        nc.vector.tensor_scalar(
                            out=it[:],
                            in0=wib[:],
                            scalar1=iotap_sb[:, wb:wb + 1],
                            scalar2=None,
                            op0=mybir.AluOpType.is_equal,
                            op1=mybir.AluOpType.add,
                            accum_out=cnt_sb[:, si * NWB + wb:si * NWB + wb + 1],
                        )
                        indT_sb[(si, wb)] = it  # noqa (kept for M3)
                # counts -> reciprocals [P, NWB]
                cntw_sb = pl_small.tile([P, NWB], F32, tag="cntw",
                                        name=f"cntw_{s}")
                for wb in range(NWB):
                    nc.vector.tensor_reduce(
                        out=cntw_sb[:, wb:wb + 1],
                        in_=cnt_sb[:, wb::NWB],
                        axis=mybir.AxisListType.X,
                        op=mybir.AluOpType.add,
                    )
                rec_sb = pl_small.tile([P, NWB], F32, tag="rec",
                                       name=f"rec_{s}")
                nc.vector.tensor_scalar_max(cntw_sb[:], cntw_sb[:], 1.0)
                nc.vector.reciprocal(rec_sb[:], cntw_sb[:])

                # ymean[wb] [128, 4] = transpose(ysumT slice) * rec
                ymean_sb = []
                for wb in range(NWB):
                    tp2 = pp_aux.tile([P, 4], F32, tag="aux",
                                        name=f"tp2_{s}_{wb}")
                    nc.tensor.transpose(
                        out=tp2[:],
                        in_=ysumT_sb[:, wb * P:(wb + 1) * P],
                        identity=ident[0:4, 0:4],
                    )
                    ym = pl_small.tile([P, 4], F32R, tag=f"ymean{wb}",
                                       name=f"ymean_{s}_{wb}")
                    nc.vector.tensor_scalar(
                        out=ym[:],
                        in0=tp2[:],
                        scalar1=rec_sb[:, wb:wb + 1],
                        scalar2=None,
                        op0=mybir.AluOpType.mult,
                    )
                    ymean_sb.append(ym)

                # gather: outT[c, s] = sum_w ymean[w, c] * indT[w, s] (+bias)
                out_sb = pl_out.tile([4, S], F32, tag="out",
                                     name=f"out_sb_{s}")
                for si in range(NST):
                    outT = pp_aux.tile([4, 512], F32, tag="aux",
                                         name=f"outT_{s}_{si}")
                    for j, wb in enumerate(strip_wbs[si]):
                        nc.tensor.matmul(
                            out=outT[:],
                            lhsT=ymean_sb[wb][:],
                            rhs=indT_sb[(si, wb)][:],
                            start=(j == 0),
                            stop=(j == len(strip_wbs[si]) - 1),
                        )
                    nc.scalar.activation(
                        out=out_sb[:, si * 512:(si + 1) * 512],
                        in_=outT[:],
                        func=mybir.ActivationFunctionType.Identity,
                        bias=bb_sb[:],
                    )
                nc.sync.dma_start(out=yout[s], in_=out_sb[0:C, :])

    nc.compile()
    return nc


def kernel(x, word_ids, W, b):
    x = np.ascontiguousarray(np.asarray(x, dtype=np.float32))
    word_ids = np.asarray(word_ids, dtype=np.int32)
    W = np.asarray(W, dtype=np.float32)
    b = np.asarray(b, dtype=np.float32)

    maps = _build_maps(word_ids)
    key = repr(maps)
    if key not in _CACHE:
        _CACHE[key] = _build_program(maps)
    nc = _CACHE[key]

    wif = word_ids.astype(np.float32)
    iota = np.broadcast_to(np.arange(NW, dtype=np.float32), (P, NW)).copy()
    iotap = (np.arange(P, dtype=np.float32)[:, None]
             + P * np.arange(NWB, dtype=np.float32)[None, :]).copy()
    wt = np.zeros((P, NHC * C), dtype=np.float32)
    for hc in range(NHC):
        wt[:, hc * C:(hc + 1) * C] = W[hc * P:(hc + 1) * P, :]
    bb = np.zeros((4, 1), dtype=np.float32)
    bb[:C, 0] = b

    in_maps = []
    for core in range(N_CORES):
        sl = slice(core * SPC, (core + 1) * SPC)
        wi_core = wif[sl]                                   # [SPC, S]
        wic = np.zeros((P, SPC * NCH), dtype=np.float32)
        for s in range(SPC):
            for ci in range(NCH):
                wic[:, s * NCH + ci] = wi_core[s, ci * P:(ci + 1) * P]
        in_maps.append({
            "xc": x[sl].reshape(SPC * S, H),
            "wic": wic,
            "wir": wi_core.reshape(1, -1).copy(),
            "iota": iota,
            "iotap": iotap,
            "wt": wt,
            "bb": bb,
            "onesd": np.ones((1, 128), dtype=np.float32),
        })

    global LAST_RESULTS
    res = run_bass_kernel_spmd(nc, in_maps, list(range(N_CORES)), trace=TRACE)
    LAST_RESULTS = res
    out = np.empty((B, S, C), dtype=np.float32)
    for core in range(N_CORES):
        yc = res.results[core]["yout"]                      # [SPC, C, S]
        out[core * SPC:(core + 1) * SPC] = yc.transpose(0, 2, 1)
    return out



# revision 13
# speedup vs baseline: 1.2331x; 1.2331x over previous
"""Trainium2 Bass kernel for per-sample segment-mean + tiny GEMM.

Reference computation (per sample b):
    sums[w]  = segment_sum(x[b], word_ids[b])           # [512, 1024]
    cnt[w]   = segment_sum(ones, word_ids[b])           # [512]
    means    = sums / max(cnt, 1)
    out[b]   = means[word_ids[b]] @ W + b               # [2048, 3]

Device strategy (data parallel: 2 samples per core, 8 cores), v2:
  A. Per 128-token chunk: ind[s,w] = (iota[w] == word_id[s]) via DVE
     tensor_scalar(is_equal).  sums[wblock] += ind[:,wblock].T @ x_chunk
     accumulated in PSUM over chunks (word blocks of 128; per-chunk
     touched word blocks computed on host from the sorted ids, unioned
     across samples so the program is SPMD-identical).
  B. NO transposes: retired sums blocks are evacuated PSUM->SBUF as
     bf16 by the scalar engine, then ysum[w,c] = sum_h sums[w,h]*W[h,c]
     via 3 DVE tensor_tensor_reduce ops per block (W pre-broadcast to
     all 128 partitions, bf16).  ymean[w,c] = ysum * (1/max(cnt,1))
     lands directly in gather-ready [w, c] layout.
  C. Transposed indicators indT[w,s] built by the GPSIMD engine from a
     ones-matmul broadcast of the ids (wib), with counts accumulated on
     the fly (accum_out).  outT[c,s] = sum_w ymean[w,c] * indT[w,s]
     (+bias) via accumulated bf16 matmuls.  Host transposes
     [3,2048] -> [2048,3] per sample.

  Engine budget per core: DMA ~53us (x stream, the floor), PE ~20us,
  DVE ~25us, GPSIMD ~21us, Scalar ~15us.  x DMA issues ride the sync
  queue; evac+bias on scalar; ind builds on DVE; indT builds on gpsimd.
"""

import numpy as np

import concourse.bass as bass
import concourse.bacc as bacc
import concourse.mybir as mybir
import concourse.tile as tile
from concourse.bass_utils import run_bass_kernel_spmd

B, S, H, C = 16, 2048, 1024, 3
NW = 512
P = 128
N_CORES = 8
SPC = B // N_CORES          # samples per core
NCH = S // P                # 128-token chunks per sample
NST = S // 512              # 512-token strips per sample
NWB = NW // P               # word blocks
F32 = mybir.dt.float32
F32R = mybir.dt.float32r
F16 = mybir.dt.float16

_CACHE = {}
TRACE = False          # set by test harness to capture an NTFF profile
LAST_RESULTS = None    # BassKernelResults of the most recent run


def _build_maps(word_ids):
    """Per-chunk / per-strip touched word-block sets, unioned across all
    samples so the same program is valid on every core (untouched blocks
    just accumulate zeros)."""
    chunk_wbs = [set() for _ in range(NCH)]
    strip_wbs = [set() for _ in range(NST)]
    for bi in range(B):
        for ci in range(NCH):
            seg = word_ids[bi, ci * P:(ci + 1) * P]
            lo, hi = int(seg.min()) // P, int(seg.max()) // P
            chunk_wbs[ci].update(range(lo, hi + 1))
        for si in range(NST):
            seg = word_ids[bi, si * 512:(si + 1) * 512]
            lo, hi = int(seg.min()) // P, int(seg.max()) // P
            strip_wbs[si].update(range(lo, hi + 1))
    # ensure every word block is touched by at least one chunk so its sums
    # region is always initialized (never-gathered garbage would still
    # poison downstream matmuls as NaN otherwise)
    seen = set().union(*chunk_wbs)
    for wb in range(NWB):
        if wb not in seen:
            chunk_wbs[0].add(wb)
    chunk_wbs = [sorted(sset) for sset in chunk_wbs]
    strip_wbs = [sorted(sset) for sset in strip_wbs]
    first_ch = {wb: min(ci for ci in range(NCH) if wb in chunk_wbs[ci])
                for wb in range(NWB)}
    last_ch = {wb: max(ci for ci in range(NCH) if wb in chunk_wbs[ci])
               for wb in range(NWB)}
    return chunk_wbs, strip_wbs, first_ch, last_ch


def _build_program(maps):
    chunk_wbs, strip_wbs, first_ch, last_ch = maps
    max_nb = max(len(wbs) for wbs in chunk_wbs)
    nc = bacc.Bacc(
        "TRN2",
        target_bir_lowering=False,
        debug=False,
        enable_asserts=False,
        num_devices=N_CORES,
    )

    # f32r DRAM tensors: fp32r memory bytes are identical to fp32, so
    # plain HWDGE loads work.
    xc = nc.dram_tensor("xc", [SPC * S, H], F32R, kind="ExternalInput").ap()
    wic = nc.dram_tensor("wic", [P, SPC * NCH], F32, kind="ExternalInput").ap()
    wir = nc.dram_tensor("wir", [1, SPC * S], F16, kind="ExternalInput").ap()
    iota = nc.dram_tensor("iota", [P, NW], F32, kind="ExternalInput").ap()
    iotap = nc.dram_tensor("iotap", [P, NWB], F32, kind="ExternalInput").ap()
    w1b = nc.dram_tensor("w1b", [1, C * H], F16, kind="ExternalInput").ap()
    bb = nc.dram_tensor("bb", [4, 1], F32, kind="ExternalInput").ap()
    yout = nc.dram_tensor("yout", [SPC, C, S], F32, kind="ExternalOutput").ap()

    XCH = 2                     # 128-token chunks per x DMA (1 MB loads)
    XG = NCH // XCH             # x DMA groups per sample

    with tile.TileContext(nc) as tc:
        with (
            tc.tile_pool(name="pp_sums", bufs=3, space="PSUM") as pp_sums,
            tc.tile_pool(name="pp_out", bufs=2, space="PSUM") as pp_out,
            tc.tile_pool(name="pl_x", bufs=6) as pl_x,
            tc.tile_pool(name="pl_ind", bufs=2 * NCH) as pl_ind,
            tc.tile_pool(name="pl_sums", bufs=3) as pl_sums,
            tc.tile_pool(name="pl_scr", bufs=1) as pl_scr,
            tc.tile_pool(name="pl_indT", bufs=26) as pl_indT,
            tc.tile_pool(name="pl_small", bufs=8) as pl_small,
            tc.tile_pool(name="pl_wib", bufs=2) as pl_wib,
            tc.tile_pool(name="pl_out", bufs=2) as pl_out,
            tc.tile_pool(name="pl_const", bufs=1) as pl_const,
        ):
            x_tiles = {}

            def load_x(s, g):
                t = pl_x.tile([P, XCH * H], F32R, tag="x", name=f"x_{s}_{g}")
                r0 = s * S + g * XCH * P
                nc.sync.dma_start(
                    out=t[:].rearrange("p (n h) -> p n h", n=XCH),
                    in_=xc[r0:r0 + XCH * P, :].rearrange(
                        "(n p) h -> p n h", p=P),
                )
                x_tiles[(s, g)] = t

            # prefetch the first x tiles before anything else so the HBM
            # stream starts at t=0
            load_x(0, 0)
            load_x(0, 1)

            # --- constants, loaded once ---
            wic_sb = pl_const.tile([P, SPC * NCH], F32, tag="wic")
            nc.sync.dma_start(out=wic_sb[:], in_=wic[:])
            wir_sb = pl_const.tile([1, SPC * S], F16, tag="wir")
            nc.sync.dma_start(out=wir_sb[:], in_=wir[:])
            iota_sb = pl_const.tile([P, NW], F32, tag="iota")
            nc.sync.dma_start(out=iota_sb[:], in_=iota[:])
            iotap_sb = pl_const.tile([P, NWB], F32, tag="iotap")
            nc.sync.dma_start(out=iotap_sb[:], in_=iotap[:])
            w1b_sb = pl_const.tile([1, C * H], F16, tag="w1b")
            nc.sync.dma_start(out=w1b_sb[:], in_=w1b[:])
            bb_sb = pl_const.tile([4, 1], F32, tag="bb")
            nc.sync.dma_start(out=bb_sb[:], in_=bb[:])


            # rest of the x stream (DMA self-flow-controls via pool bufs)
            for g in range(2, XG):
                load_x(0, g)
            for g in range(XG):
                load_x(1, g)

            # W broadcast to all partitions: [128, C*H] bf16
            wb_sb = pl_const.tile([P, C * H], F16, tag="wb")
            nc.gpsimd.partition_broadcast(wb_sb[:], w1b_sb[0:1, :])

            indT_sb = {}
            for s in range(SPC):
                # ---- indT builds (only need wir) ----
                # wib[si] = ids broadcast across partitions (PE
                # ones-matmul), indT[w,s] = is_equal(wib, iota_part);
                # counts accumulate on the fly into cnt_sb columns.
                cnt_sb = pl_small.tile([P, NST * NWB], F32, tag="cnt",
                                       name=f"cnt_{s}")
                nc.vector.memset(cnt_sb[:], 0.0)
                for si in range(NST):
                    wib = pl_wib.tile([P, 512], F16, tag="wib",
                                      name=f"wib_{s}_{si}")
                    nc.gpsimd.partition_broadcast(
                        wib[:],
                        wir_sb[0:1, s * S + si * 512:s * S + (si + 1) * 512],
                    )
                    for wb in strip_wbs[si]:
                        it = pl_indT.tile([P, 512], F16, tag="indT",
                                          name=f"indT_{s}_{si}_{wb}")
                        nc.vector.tensor_scalar(
                            out=it[:],
                            in0=wib[:],
                            scalar1=iotap_sb[:, wb:wb + 1],
                            scalar2=None,
                            op0=mybir.AluOpType.is_equal,
                            op1=mybir.AluOpType.add,
                            accum_out=cnt_sb[
                                :, si * NWB + wb:si * NWB + wb + 1],
                        )
                        indT_sb[(s, si, wb)] = it
                # ---- A-phase indicators (DVE; only need consts) ----
                ind_t = {}
                for ci in range(NCH):
                    lo, hi = chunk_wbs[ci][0], chunk_wbs[ci][-1]
                    nb = hi - lo + 1
                    ind = pl_ind.tile([P, max_nb * P], F32R, tag="ind",
                                      name=f"ind_{s}_{ci}")
                    nc.vector.tensor_scalar(
                        out=ind[:, 0:nb * P],
                        in0=iota_sb[:, lo * P:(hi + 1) * P],
                        scalar1=wic_sb[:, s * NCH + ci:s * NCH + ci + 1],
                        scalar2=None,
                        op0=mybir.AluOpType.is_equal,
                    )
                    ind_t[ci] = ind

                # counts -> reciprocals [P, NWB] (after this sample's indT)
                cntw_sb = pl_small.tile([P, NWB], F32, tag="cntw",
                                        name=f"cntw_{s}")
                for wb in range(NWB):
                    nc.vector.tensor_reduce(
                        out=cntw_sb[:, wb:wb + 1],
                        in_=cnt_sb[:, wb::NWB],
                        axis=mybir.AxisListType.X,
                        op=mybir.AluOpType.add,
                    )
                rec_sb = pl_small.tile([P, NWB], F32, tag="rec",
                                       name=f"rec_{s}")
                nc.vector.tensor_scalar_max(cntw_sb[:], cntw_sb[:], 1.0)
                nc.vector.reciprocal(rec_sb[:], cntw_sb[:])

                # ---------------- Phase A: segment sums ----------------
                sums_ps = {}
                ymean_sb = {}
                for ci in range(NCH):
                    x4 = x_tiles[(s, ci // XCH)]
                    xv = x4[:, (ci % XCH) * H:(ci % XCH + 1) * H]
                    lo = chunk_wbs[ci][0]
                    ind = ind_t[ci]
                    for wb in chunk_wbs[ci]:
                        if ci == first_ch[wb]:
                            sums_ps[wb] = pp_sums.tile(
                                [P, H], F32, tag="sums",
                                name=f"sums_{s}_{wb}")
                        for hh in range(2):
                            nc.tensor.matmul(
                                out=sums_ps[wb][:, hh * 512:(hh + 1) * 512],
                                lhsT=ind[:, (wb - lo) * P:(wb - lo + 1) * P],
                                rhs=xv[:, hh * 512:(hh + 1) * 512],
                                start=(ci == first_ch[wb]),
                                stop=(ci == last_ch[wb]),
                            )
                    # retire finished word blocks: evac (bf16) + ysum + ymean
                    for wb in list(sums_ps.keys()):
                        if ci != last_ch[wb]:
                            continue
                        sums_sb = pl_sums.tile([P, H], F16, tag="sums_sb",
                                               name=f"sums_sb_{s}_{wb}")
                        nc.scalar.copy(out=sums_sb[:], in_=sums_ps[wb][:])
                        del sums_ps[wb]
                        # ymean[w,c] = sum_h (sums[w,h]*rec[w]) * W[h,c],
                        # fully fused per c on the DVE
                        scr = pl_scr.tile([P, H], F16, tag="scr")
                        ymr = pl_small.tile([P, 4], F32, tag="ymr",
                                            name=f"ymr_{s}_{wb}")
                        nc.vector.memset(ymr[:], 0.0)
                        for c in range(C):
                            nc.vector.scalar_tensor_tensor(
                                out=scr[:],
                                in0=sums_sb[:],
                                scalar=rec_sb[:, wb:wb + 1],
                                in1=wb_sb[:, c * H:(c + 1) * H],
                                op0=mybir.AluOpType.mult,
                                op1=mybir.AluOpType.mult,
                                accum_out=ymr[:, c:c + 1],
                            )
                        ym = pl_small.tile([P, 4], F16, tag="ymean",
                                           name=f"ymean_{s}_{wb}")
                        nc.vector.tensor_copy(out=ym[:], in_=ymr[:])
                        ymean_sb[wb] = ym

                # ---------------- Phase C: gather ----------------------
                out_sb = pl_out.tile([4, S], F32, tag="out",
                                     name=f"out_sb_{s}")
                for si in range(NST):
                    outT = pp_out.tile([4, 512], F32, tag="outT",
                                       name=f"outT_{s}_{si}")
                    for j, wb in enumerate(strip_wbs[si]):
                        nc.tensor.matmul(
                            out=outT[:],
                            lhsT=ymean_sb[wb][:],
                            rhs=indT_sb[(s, si, wb)][:],
                            start=(j == 0),
                            stop=(j == len(strip_wbs[si]) - 1),
                        )
                    nc.scalar.activation(
                        out=out_sb[:, si * 512:(si + 1) * 512],
                        in_=outT[:],
                        func=mybir.ActivationFunctionType.Identity,
                        bias=bb_sb[:],
                    )
                nc.scalar.dma_start(out=yout[s], in_=out_sb[0:C, :])

    nc.compile()
    return nc


def _host_inputs(x, word_ids, W, b):
    """Per-core input maps (shared by kernel() and the test's sim path)."""
    wif = word_ids.astype(np.float32)
    iota = np.broadcast_to(np.arange(NW, dtype=np.float32), (P, NW)).copy()
    iotap = (np.arange(P, dtype=np.float32)[:, None]
             + P * np.arange(NWB, dtype=np.float32)[None, :]).copy()
    w1b = np.ascontiguousarray(W.T.reshape(1, C * H)).astype(
        np.float16)
    bb = np.zeros((4, 1), dtype=np.float32)
    bb[:C, 0] = b

    in_maps = []
    for core in range(N_CORES):
        sl = slice(core * SPC, (core + 1) * SPC)
        wi_core = wif[sl]                                   # [SPC, S]
        wic = np.zeros((P, SPC * NCH), dtype=np.float32)
        for s in range(SPC):
            for ci in range(NCH):
                wic[:, s * NCH + ci] = wi_core[s, ci * P:(ci + 1) * P]
        in_maps.append({
            "xc": x[sl].reshape(SPC * S, H),
            "wic": wic,
            "wir": wi_core.reshape(1, -1).astype(np.float16),
            "iota": iota,
            "iotap": iotap,
            "w1b": w1b,
            "bb": bb,
        })
    return in_maps


def kernel(x, word_ids, W, b):
    x = np.ascontiguousarray(np.asarray(x, dtype=np.float32))
    word_ids = np.asarray(word_ids, dtype=np.int32)
    W = np.asarray(W, dtype=np.float32)
    b = np.asarray(b, dtype=np.float32)

    maps = _build_maps(word_ids)
    key = repr(maps)
    if key not in _CACHE:
        _CACHE[key] = _build_program(maps)
    nc = _CACHE[key]

    in_maps = _host_inputs(x, word_ids, W, b)

    global LAST_RESULTS
    res = run_bass_kernel_spmd(nc, in_maps, list(range(N_CORES)), trace=TRACE)
    LAST_RESULTS = res
    out = np.empty((B, S, C), dtype=np.float32)
    for core in range(N_CORES):
        yc = res.results[core]["yout"]                      # [SPC, C, S]
        out[core * SPC:(core + 1) * SPC] = yc.transpose(0, 2, 1)
    return out
